# revision 5
# baseline (speedup 1.0000x reference)
"""Trainium2 Bass kernel for GPLinear (geometric-product linear layer, Cl(3,0)).

    out[b,o,k] = sum_{i,j,p} G[i,j,k] * x[b,p,i] * W[p,o,j] + bias[o,k]

G is the Cl(3,0) Cayley tensor: G[i,j,k] != 0 only at k = i^j with value
sign(i,j) = +-1. So for each output blade k:

    out[:,:,k] = sum_i sign(i, i^k) * x[:,:,i] @ W[:,:,i^k]

Device mapping (per core):
  - PE-transpose x[b,p,i] -> xT_i[p,b] (f32r transpose-mode matmuls)
  - store [+W; -W] in SBUF; matmul rhs reads it through XOR-structured
    access patterns that realize j = i^k and the sign in the address.
    PSUM column layout (k1, o, k2, k0): fixing k1 per matmul chunk makes
    every blade's sign pattern AP-affine and every chunk a contiguous,
    bank-aligned N=512 block.
  - evacuation tensor_tensor un-permutes to (o,k) and adds broadcast bias.
Matmuls run in float32r (FP22 mantissa-13) at full PE rate.

Sharding (8 cores): 4-way over batch x 2-way over out_features.
core c -> batch rows [ (c//2)*128, +128 ), out cols [ (c%2)*256, +256 ).
"""

import numpy as np

import concourse.bass as bass
import concourse.mybir as mybir
import concourse.tile as tile
from concourse import bacc
from concourse.bass_utils import run_bass_kernel_spmd
from concourse.masks import make_identity

F32 = mybir.dt.float32
F32R = mybir.dt.float32r

# problem sizes (hardcoded per spec: x[512,512,8], W[512,512,8], b[512,8])
BATCH, IN_F, OUT_F, K8 = 512, 512, 512, 8
R_B, R_O = 4, 2                 # sharding factors (batch x out_features)
N_CORES = R_B * R_O
B_LOC = BATCH // R_B            # 128 batch rows per core
OC = OUT_F // R_O               # 256 out features per core
PT = IN_F // 128                # 4 p-tiles
O_SUB = 128                     # o's per matmul chunk (N = O_SUB*4 = 512)

LAST_RESULTS = None             # BassKernelResults of the most recent run


def _sign_structure(G):
    """Per blade i: (r0, B) such that  sign(i, i^k) < 0  <=>  r0 ^ XOR_{b in B} k_b.

    Verifies G has the Cl(3,0) XOR-sparsity + affine sign structure."""
    G = np.asarray(G, dtype=np.float32)
    assert G.shape == (8, 8, 8)
    for i in range(8):
        for j in range(8):
            for k in range(8):
                if k != (i ^ j):
                    assert G[i, j, k] == 0.0, "G not XOR-sparse"
    out = []
    for i in range(8):
        s = np.array([G[i, i ^ k, k] for k in range(8)])
        assert np.all(np.abs(s) == 1.0), "G signs not +-1"
        r = (s < 0).astype(int)
        r0 = int(r[0])
        B = tuple(b for b in range(3) if r[1 << b] != r0)
        for k in range(8):
            pred = r0
            for b in B:
                pred ^= (k >> b) & 1
            assert pred == r[k], "G sign pattern not XOR-affine"
        out.append((r0, B))
    return tuple(out)


def _build(signs, loop_n=None):
    nc = bacc.Bacc("TRN2", target_bir_lowering=False, debug=False)

    x_d = nc.dram_tensor("x", [B_LOC, IN_F, K8], F32, kind="ExternalInput")
    w_d = nc.dram_tensor("w", [IN_F, OC, K8], F32, kind="ExternalInput")
    b_d = nc.dram_tensor("b", [OC, K8], F32, kind="ExternalInput")
    o_d = nc.dram_tensor("out", [B_LOC, OC, K8], F32, kind="ExternalOutput")

    D = OC * K8  # +W/-W region offset in elements
    import contextlib

    with tile.TileContext(nc) as tc:
        with (
            tc.tile_pool(name="sb", bufs=1) as sb,
            tc.tile_pool(name="ps", bufs=1, space="PSUM") as ps,
            tc.tile_pool(name="trp", bufs=2, space="PSUM") as trp,
            (tc.For_i(0, loop_n, 1) if loop_n else contextlib.nullcontext()),
        ):
            x_sb = sb.tile([128, PT * 128 * K8], F32R, tag="x")
            ww = [sb.tile([128, 2, D], F32R, tag=f"ww{t}", name=f"ww{t}")
                  for t in range(PT)]
            # xT layout [p, t, i, b] so per-t evacuation is contiguous
            xT = sb.tile([128, PT, K8, B_LOC], F32R, tag="xT")
            bias_sb = sb.tile([128, D], F32, tag="bias")
            out_sb = sb.tile([128, D], F32, tag="out")
            ident_f = sb.tile([128, 128], F32, tag="identf")
            ident = sb.tile([128, 128], F32R, tag="ident")
            acc = ps.tile([128, D], F32, tag="acc")  # 2048 cols = 4 banks

            make_identity(nc, ident_f[:])
            nc.vector.tensor_copy(ident[:], ident_f[:])

            x_r = x_sb[:].rearrange("b (p k) -> b p k", k=K8)

            # ---- interleaved loads + negate + transpose, per p-tile ----
            for t in range(PT):
                # x chunk t, then W chunk t (DMA queue processes in order)
                nc.sync.dma_start(
                    x_sb[:, t * 128 * K8:(t + 1) * 128 * K8],
                    x_d.ap()[:, t * 128:(t + 1) * 128, :]
                       .rearrange("b p k -> b (p k)").bitcast(F32R))
                nc.sync.dma_start(
                    ww[t][:, 0, :],
                    w_d.ap()[t * 128:(t + 1) * 128]
                       .rearrange("p o k -> p (o k)").bitcast(F32R))
                nc.vector.tensor_scalar(
                    out=ww[t][:, 1, :], in0=ww[t][:, 0, :],
                    scalar1=-1.0, scalar2=None, op0=mybir.AluOpType.mult)

                # transposes of all 8 blades for this p-tile
                tr = trp.tile([128, K8 * 128], F32R, tag="tr")  # 2 banks
                for i in range(K8):
                    nc.tensor.transpose(
                        tr[:, i * 128:(i + 1) * 128],
                        x_r[:, t * 128:(t + 1) * 128, i],
                        ident[:])
                tr_r = tr[:].rearrange("p (i b) -> p i b", i=K8)
                for h in range(2):
                    dst = xT[:, t, h * 4:(h + 1) * 4, :]
                    srcv = tr_r[:, h * 4:(h + 1) * 4, :]
                    if h == 0:
                        nc.scalar.copy(dst, srcv)
                    else:
                        nc.vector.tensor_copy(dst, srcv)

            nc.sync.dma_start(
                bias_sb[:],
                b_d.ap().rearrange("o k -> (o k)").unsqueeze(0)
                   .partition_broadcast(128))

            # ---- matmuls (t-outer: overlap with W DMA pipeline) ----
            pitch_ww = ww[0][:].ap[0][0]
            pitch_acc = acc[:].ap[0][0]

            def r_of(i, k):
                r0, Bset = signs[i]
                v = r0
                for b in Bset:
                    v ^= (k >> b) & 1
                return v

            def emit(i, t, v, o0, start, stop):
                r0, Bset = signs[i]
                k_base = v << 1
                j_base = i ^ k_base
                r_base = r_of(i, k_base)
                dims = [[pitch_ww, 128], [8, O_SUB]]
                for b in (2, 0):
                    step = (1 - 2 * ((i >> b) & 1)) * (1 << b)
                    if b in Bset:
                        step += (1 - 2 * r_base) * D
                    dims.append([step, 2])
                rhs = bass.AP(tensor=ww[t].tensor,
                              offset=r_base * D + o0 * 8 + j_base,
                              ap=dims)
                out_ap = bass.AP(tensor=acc.tensor, offset=v * OC * 4 + o0 * 4,
                                 ap=[[pitch_acc, 128], [1, O_SUB * 4]])
                nc.tensor.matmul(out_ap, xT[:, t, i, :], rhs,
                                 start=start, stop=stop)

            for t in range(PT):
                for i in range(K8):
                    for v in range(2):
                        for os_ in range(OC // O_SUB):
                            emit(i, t, v, os_ * O_SUB,
                                 start=(t == 0 and i == 0),
                                 stop=(t == PT - 1 and i == K8 - 1))

            # ---- per-bank un-permute (k1,o,k2,k0) -> (o,k) + bias ----
            n_os = OC // O_SUB
            for v in range(2):
                for os_ in range(n_os):
                    def perm_ap(tl):
                        return bass.AP(
                            tensor=tl.tensor,
                            offset=os_ * O_SUB * 8 + 2 * v,
                            ap=[[tl[:].ap[0][0], 128], [8, O_SUB], [4, 2], [1, 2]])
                    in0 = bass.AP(tensor=acc.tensor,
                                  offset=v * OC * 4 + os_ * O_SUB * 4,
                                  ap=[[pitch_acc, 128], [1, O_SUB * 4]])
                    nc.vector.tensor_tensor(out=perm_ap(out_sb), in0=in0,
                                            in1=perm_ap(bias_sb),
                                            op=mybir.AluOpType.add)

            # ---- store (split by o-half to overlap with evacuation) ----
            for os_ in range(n_os):
                nc.sync.dma_start(
                    o_d.ap()[:, os_ * O_SUB:(os_ + 1) * O_SUB, :]
                       .rearrange("b o k -> b (o k)"),
                    out_sb[:, os_ * O_SUB * 8:(os_ + 1) * O_SUB * 8])

    nc.compile()
    return nc


_CACHE = {}


def kernel(x, W, b, G):
    global LAST_RESULTS
    x = np.ascontiguousarray(np.asarray(x, dtype=np.float32))
    W = np.asarray(W, dtype=np.float32)
    b = np.asarray(b, dtype=np.float32)
    signs = _sign_structure(G)

    if signs not in _CACHE:
        _CACHE[signs] = _build(signs)
    nc = _CACHE[signs]

    in_maps = []
    for c in range(N_CORES):
        bc, oc = divmod(c, R_O)
        in_maps.append({
            "x": np.ascontiguousarray(x[bc * B_LOC:(bc + 1) * B_LOC]),
            "w": np.ascontiguousarray(W[:, oc * OC:(oc + 1) * OC, :]),
            "b": np.ascontiguousarray(b[oc * OC:(oc + 1) * OC]),
        })

    res = run_bass_kernel_spmd(nc, in_maps, core_ids=list(range(N_CORES)))
    LAST_RESULTS = res

    out = np.empty((BATCH, OUT_F, K8), dtype=np.float32)
    for c in range(N_CORES):
        bc, oc = divmod(c, R_O)
        out[bc * B_LOC:(bc + 1) * B_LOC, oc * OC:(oc + 1) * OC, :] = \
            res.results[c]["out"]
    return out
